# revision 39
# baseline (speedup 1.0000x reference)
"""Multi-head causal attention (B=4, T=2048, C=1024, H=16) on 8 TRN2 cores.

Sharding: core i handles batch b = i//2 and head-group g = i%2 (8 heads each).
Each core computes qkv projection for its heads, causal attention, and a
partial output projection (its heads' rows of W_o). The host sums the two
partials per batch and adds b_o (plus b_v @ W_o — the v-bias contribution is
linear and folded into the host-side bias; the k-bias is dropped entirely
since a per-query multiplicative factor cancels in the softmax).

Device kernel (per core, same SPMD program), all matmuls bf16 with fp32 PSUM.
Single merged pipeline: the qkv/o projections are emitted as PE "filler"
units interleaved between attention segments, so the scalar engine (exp) gets
work from ~8us onward and the tensor engine never idles between phases.

  - qkT = (Wqk^T x^T) transposed [1024 feats, 2048] computed in 32 units of
    (nt feature-block, tt 512-query-block), each 8 cc-chunk matmuls; q units
    get the q-bias on DVE, k units are plain copies
  - v = x Wv natural [2048, 512] in 16 per-t-tile units
  - per head pair (2hp, 2hp+1), per 512-wide query block tb, per 128-wide key
    chunk j (causal-trimmed):
      S^T = K^T q^T   [128 tk, tq]  (row-packed pairs, K=64 contraction)
      P^T = exp(S^T * 0.125)        (ACT; diagonal chunks masked on Pool)
      AV^T += [V | 1]^T P^T         [65, tq]  (row 64 = softmax denominator)
    scores for segment s+1 are chunk-interleaved with AV of segment s;
    reciprocals fire straight off the PSUM denominator row at AV-complete;
    normalization (ones-outer-product broadcast on PE, then a fused
    PSUM-direct multiply on DVE) lags two segments
  - out_part = attT^T W_o rows [2048, 1024], per query-block group as soon as
    its last head pair normalizes; PSUM drained on DVE, stored bf16
  - PSUM: score pipeline 2x[128,1024] (4 banks), AV accumulators 2x[128,512]
    (2 banks), shared fill tag for proj/oproj/norm 2x[128,512] (2 banks)
"""

import sys

sys.path.insert(0, "/opt/trn_rl_repo")

import numpy as np
import ml_dtypes

BF16 = ml_dtypes.bfloat16

B, T, C, H, D = 4, 2048, 1024, 16, 64
HPC = 8        # heads per core
CQ = HPC * D   # 512
NCORES = 8
P = 128


def _split_waits(nc):
    """This container's walrus accepts only ONE sync wait per instruction.
    Split any instruction carrying N>1 waits into N-1 single-wait NoOps on
    the same engine immediately before it."""
    import concourse.mybir as mybir

    ctr = 0
    for fn in nc.m.functions:
        for bb in fn.blocks:
            insts = list(bb.instructions)
            new_insts = []
            changed = False
            for inst in insts:
                si = inst.sync_info
                if si is not None and si.on_wait and len(si.on_wait) > 1:
                    waits = list(si.on_wait)
                    for w in waits[:-1]:
                        ctr += 1
                        nop = mybir.InstNoOp(
                            name=f"I-wsplit-{ctr}",
                            engine=inst.engine,
                            ins=[],
                            outs=[],
                            sync_info=mybir.SyncInfo(on_wait=[w], on_update=[]),
                        )
                        new_insts.append(nop)
                    si.on_wait = [waits[-1]]
                    changed = True
                new_insts.append(inst)
            if changed:
                bb.instructions[:] = new_insts
    return ctr


def _declare(nc):
    import concourse.mybir as mybir

    bf = mybir.dt.bfloat16
    f32 = mybir.dt.float32
    return dict(
        xT=nc.dram_tensor("xT", [C, T], bf, kind="ExternalInput").ap(),
        wqk=nc.dram_tensor("wqk", [C, 2 * CQ], bf, kind="ExternalInput").ap(),
        bq=nc.dram_tensor("bq", [P, 4], f32, kind="ExternalInput").ap(),
        wv=nc.dram_tensor("wv", [C, CQ], bf, kind="ExternalInput").ap(),
        wo=nc.dram_tensor("wo", [CQ, C], bf, kind="ExternalInput").ap(),
        maskT=nc.dram_tensor("maskT", [P, P], bf, kind="ExternalInput").ap(),
        ones1=nc.dram_tensor("ones1", [1, 64], bf, kind="ExternalInput").ap(),
        outp=nc.dram_tensor("outp", [T, C], bf, kind="ExternalOutput").ap(),
        # hc=3 contribution of the last query-block group (rows 1536:2048);
        # the host adds it — keeps the kernel tail free of partial-add chains
        outp2=nc.dram_tensor("outp2", [512, C], bf, kind="ExternalOutput").ap(),
    )


def _emit(nc, tc, aps):
    import concourse.mybir as mybir
    from concourse.alu_op_type import AluOpType

    bf = mybir.dt.bfloat16
    f32 = mybir.dt.float32
    Exp = mybir.ActivationFunctionType.Exp

    xT = aps["xT"]; wqk = aps["wqk"]; bq = aps["bq"]; wv = aps["wv"]
    wo = aps["wo"]; maskT = aps["maskT"]; ones1 = aps["ones1"]
    outp = aps["outp"]; outp2 = aps["outp2"]

    NTT = T // 512  # 4 query blocks
    VW = HPC * 65   # 520: v row layout (64 cols + ones col per head)

    with tc.tile_pool(name="const", bufs=1) as cpool, \
         tc.tile_pool(name="proj", bufs=1) as ppool, \
         tc.tile_pool(name="ps2", bufs=1, space="PSUM") as ps2, \
         tc.tile_pool(name="ps4", bufs=1, space="PSUM") as ps4, \
         tc.tile_pool(name="att", bufs=1) as apool, \
         tc.tile_pool(name="out", bufs=1) as opool:

        wo_sb = cpool.tile([P, 4 * 1024], bf)
        maskT_sb = cpool.tile([P, P], bf)
        ones1_sb = cpool.tile([1, 64], bf)
        qkT_sb = cpool.tile([P, 8 * T], bf)
        v_sb = cpool.tile([P, 16 * VW], bf)
        attn_sb = cpool.tile([P, 16 * 512], bf)     # unnorm AV^T, pair layout

        xT_sb = ppool.tile([P, 8 * T], bf)
        wqk_sb = ppool.tile([P, 8 * 1024], bf)
        wv_sb = ppool.tile([P, 8 * CQ], bf)
        bq_sb = ppool.tile([P, 4], f32)

        # DMA order: what the first proj units need comes first — k-half of
        # wqk + first query-quarter of xT (unit qk(4,0)), then the q-half
        # (unit qk(0,0)), then wv (v units follow the first s_chunks), then
        # the remaining xT quarters and wo. One rearranged descriptor per
        # logical block: HWDGE per-descriptor overhead dominates the startup
        # window otherwise.
        wqk_v = wqk_sb.rearrange("p (c n) -> p c n", c=8)
        wqk_d = wqk.rearrange("(c p) n -> p c n", p=P)
        xT_v = xT_sb.rearrange("p (c n) -> p c n", c=8)
        xT_d = xT.rearrange("(c p) n -> p c n", p=P)
        nc.sync.dma_start(wqk_v[:, :, 512:640], wqk_d[:, :, 512:640])   # nt4
        for c4 in range(4):
            nc.sync.dma_start(xT_v[:, 2 * c4:2 * c4 + 2, 0:512],
                              xT_d[:, 2 * c4:2 * c4 + 2, 0:512])
        nc.sync.dma_start(wqk_v[:, :, 0:128], wqk_d[:, :, 0:128])       # nt0
        nc.sync.dma_start(bq_sb[:], bq[:])
        nc.sync.dma_start(wqk_v[:, :, 640:1024], wqk_d[:, :, 640:1024])
        nc.sync.dma_start(wqk_v[:, :, 128:512], wqk_d[:, :, 128:512])
        nc.sync.dma_start(wv_sb.rearrange("p (c n) -> p c n", c=8),
                          wv.rearrange("(c p) n -> p c n", p=P))
        nc.sync.dma_start(maskT_sb[:], maskT[:])
        nc.sync.dma_start(ones1_sb[:], ones1[:])
        nc.sync.dma_start(xT_v[:, :, 512:2048], xT_d[:, :, 512:2048])
        nc.sync.dma_start(wo_sb.rearrange("p (h n) -> p h n", h=4),
                          wo.rearrange("(h p) n -> p h n", p=P))

        # ones columns of v (col 64 of each head's 65-wide strip)
        v_ones = v_sb.rearrange("p (a c) -> p a c", c=65)
        nc.gpsimd.memset(v_ones[:, :, 64:65], 1.0)

        # ---------------- filler units (projections) ----------------
        # generator form: one cc-chunk matmul per next(), so units can be
        # spread across the attention chunk slots — the extra PE work between
        # successive score chunks hides the scores->exp->AV semaphore
        # round-trip that would otherwise stall PE on the 2-deep score PSUM
        # rotation
        def qk_unit_gen(nt, tt):
            psq = ps4.tile([P, 512], f32, tag="fill", bufs=2,
                           name=f"psq_{nt}_{tt}")
            for cc in range(8):
                nc.tensor.matmul(
                    psq[:],
                    wqk_sb[:, cc * 1024 + nt * P: cc * 1024 + (nt + 1) * P],
                    xT_sb[:, cc * T + tt * 512: cc * T + (tt + 1) * 512],
                    start=(cc == 0),
                    stop=(cc == 7),
                )
                if cc < 7:
                    yield
            dst = qkT_sb[:, nt * T + tt * 512: nt * T + (tt + 1) * 512]
            if nt < 4:
                nc.vector.tensor_scalar(dst, psq[:], bq_sb[:, nt:nt + 1],
                                        None, op0=AluOpType.add)
            else:
                nc.vector.tensor_copy(dst, psq[:])

        def v_unit_gen(tt16):
            psv = ps4.tile([P, 512], f32, tag="fill", bufs=2,
                           name=f"psv_{tt16}")
            for cc in range(8):
                nc.tensor.matmul(
                    psv[:],
                    xT_sb[:, cc * T + tt16 * P: cc * T + (tt16 + 1) * P],
                    wv_sb[:, cc * CQ:(cc + 1) * CQ],
                    start=(cc == 0),
                    stop=(cc == 7),
                )
                if cc < 7:
                    yield
            vv = v_sb[:, tt16 * VW:(tt16 + 1) * VW].rearrange(
                "p (a c) -> p a c", c=65)
            nc.vector.tensor_copy(
                vv[:, :, 0:64], psv[:].rearrange("p (a c) -> p a c", c=64))

        def qk_unit(nt, tt):
            for _ in qk_unit_gen(nt, tt):
                pass

        def v_unit(tt16):
            for _ in v_unit_gen(tt16):
                pass

        # ---------------- attention ----------------
        pts = {}
        recs = {}

        def s_chunk(hp, tb, j):
            h0, h1 = 2 * hp, 2 * hp + 1
            off = j * P - tb * 512
            nstart = max(off, 0)
            pss = ps2.tile([P, 1024], f32, tag="pss", bufs=2,
                           name=f"pss_{hp}_{tb}_{j}")
            pt = apool.tile([P, 1024], bf, tag="pt", bufs=20,
                            name=f"pt_{hp}_{tb}_{j}")
            pts[(hp, tb, j)] = pt
            for i, hl in enumerate((h0, h1)):
                base = (hl % 2) * 64
                nc.tensor.matmul(
                    pss[:, i * 512 + nstart: i * 512 + 512],
                    qkT_sb[base:base + 64,
                           (4 + hp) * T + j * P: (4 + hp) * T + (j + 1) * P],
                    qkT_sb[base:base + 64,
                           hp * T + tb * 512 + nstart: hp * T + (tb + 1) * 512],
                    start=True,
                    stop=True,
                )
            pw = pss.rearrange("p (a c) -> p a c", c=512)
            ptw = pt.rearrange("p (a c) -> p a c", c=512)
            nc.scalar.activation(
                ptw[:, :, nstart:512], pw[:, :, nstart:512], Exp, scale=0.125,
            )
            if off >= 0:
                for i in range(2):
                    nc.gpsimd.tensor_tensor(
                        pt[:, i * 512 + nstart: i * 512 + nstart + P],
                        pt[:, i * 512 + nstart: i * 512 + nstart + P],
                        maskT_sb[:],
                        op=AluOpType.mult,
                    )

        def av_chunk(hp, tb, j, psav):
            h0, h1 = 2 * hp, 2 * hp + 1
            jmax = 4 * tb + 3
            off = j * P - tb * 512
            nstart = max(off, 0)
            for i, hl in enumerate((h0, h1)):
                nc.tensor.matmul(
                    psav[i][0:65, nstart:512],
                    v_sb[:, j * VW + hl * 65: j * VW + (hl + 1) * 65],
                    pts[(hp, tb, j)][:, i * 512 + nstart: i * 512 + 512],
                    start=(j == 0),
                    stop=(j == jmax),
                )
            if j == jmax:
                seg = hp * NTT + tb
                rece = apool.tile([1, 512], bf, tag="rec", bufs=8,
                                  name=f"rece_{seg}")
                reco = apool.tile([1, 512], bf, tag="rec", bufs=8,
                                  name=f"reco_{seg}")
                with nc.allow_low_precision(reason="bf16 softmax denominators"):
                    nc.vector.reciprocal(rece[:], psav[0][64:65, :])
                    nc.vector.reciprocal(reco[:], psav[1][64:65, :])
                recs[seg] = (rece, reco)
                nc.vector.tensor_copy(attn_sb[0:64, seg * 512:(seg + 1) * 512],
                                      psav[0][0:64, :])
                nc.vector.tensor_copy(attn_sb[64:128, seg * 512:(seg + 1) * 512],
                                      psav[1][0:64, :])

        def norm_seg(seg):
            # denominator reciprocals broadcast across partitions on PE (ones
            # outer product), then applied straight from PSUM on DVE — keeps
            # the normalize off Pool, whose in-order queue stalls on
            # exp-gated mask multiplies
            sl = slice(seg * 512, (seg + 1) * 512)
            rece, reco = recs.pop(seg)
            if seg == 15:
                # tail: score banks are idle, avoid the fill-tag rotation
                psr = ps2.tile([P, 1024], f32, tag="pss", bufs=2,
                               name=f"psr_{seg}")[:, 0:512]
            else:
                psr = ps4.tile([P, 512], f32, tag="fill", bufs=2,
                               name=f"psr_{seg}")
            nc.tensor.matmul(psr[0:64, :], ones1_sb[:], rece[:],
                             start=True, stop=True)
            nc.tensor.matmul(psr[64:128, :], ones1_sb[:], reco[:],
                             start=True, stop=True)
            nc.vector.scalar_tensor_tensor(
                attn_sb[0:64, sl], psr[0:64, :], 1.0, attn_sb[0:64, sl],
                op0=AluOpType.bypass, op1=AluOpType.mult)
            nc.vector.scalar_tensor_tensor(
                attn_sb[64:128, sl], psr[64:128, :], 1.0, attn_sb[64:128, sl],
                op0=AluOpType.bypass, op1=AluOpType.mult)

        def oproj_group(tbg):
            # output projection for query rows tbg*512 .. +512 (4 t-tiles)
            for tt16 in range(tbg * 4, tbg * 4 + 4):
                psos = [
                    ps4.tile([P, 512], f32, tag="fill", bufs=2,
                             name=f"pso_{tt16}_{mb}")
                    for mb in range(2)
                ]
                for hc in range(4):
                    seg = hc * NTT + tbg
                    col = (seg * 4 + tt16 % 4) * P
                    for mb in range(2):
                        nc.tensor.matmul(
                            psos[mb][:],
                            attn_sb[:, col: col + P],
                            wo_sb[:, hc * 1024 + mb * 512: hc * 1024 + (mb + 1) * 512],
                            start=(hc == 0),
                            stop=(hc == 3),
                        )
                for mb in range(2):
                    osb = opool.tile([P, 512], bf, tag="osb", bufs=4,
                                     name=f"osb_{tt16}_{mb}")
                    nc.vector.tensor_copy(osb[:], psos[mb][:])
                    nc.sync.dma_start(
                        outp[tt16 * P:(tt16 + 1) * P, mb * 512:(mb + 1) * 512],
                        osb[:],
                    )

        # The last group's projection would otherwise sit entirely in the
        # kernel tail (it needs the very last segment normalized). Split it:
        # head-chunks 0..2 go straight to outp while segment 15's AV still
        # runs; the tail does only the hc=3 matmuls, whose result is shipped
        # as a separate output for the host to add.
        def oproj3_partA(tt16):
            psos = [
                ps4.tile([P, 512], f32, tag="fill", bufs=2,
                         name=f"psoA_{tt16}_{mb}")
                for mb in range(2)
            ]
            for hc in range(3):
                seg = hc * NTT + 3
                col = (seg * 4 + tt16 % 4) * P
                for mb in range(2):
                    nc.tensor.matmul(
                        psos[mb][:],
                        attn_sb[:, col: col + P],
                        wo_sb[:, hc * 1024 + mb * 512: hc * 1024 + (mb + 1) * 512],
                        start=(hc == 0),
                        stop=(hc == 2),
                    )
            for mb in range(2):
                osb = opool.tile([P, 512], bf, tag="osb", bufs=4,
                                 name=f"osbA_{tt16}_{mb}")
                nc.vector.tensor_copy(osb[:], psos[mb][:])
                nc.sync.dma_start(
                    outp[tt16 * P:(tt16 + 1) * P, mb * 512:(mb + 1) * 512],
                    osb[:],
                )

        def oproj3_partB():
            # runs in the kernel tail: use the psav banks (free once segment
            # 15's AV is drained) so the chain doesn't contend with the fill
            # tag's rotation
            for tt16 in range(12, 16):
                psos = [
                    ps2.tile([P, 512], f32, tag="psav", bufs=2,
                             name=f"psoB_{tt16}_{mb}")
                    for mb in range(2)
                ]
                seg = 3 * NTT + 3
                col = (seg * 4 + tt16 % 4) * P
                for mb in range(2):
                    nc.tensor.matmul(
                        psos[mb][:],
                        attn_sb[:, col: col + P],
                        wo_sb[:, 3 * 1024 + mb * 512: 3 * 1024 + (mb + 1) * 512],
                        start=True,
                        stop=True,
                    )
                for mb in range(2):
                    osb = opool.tile([P, 512], bf, tag="osb", bufs=4,
                                     name=f"osbB_{tt16}_{mb}")
                    nc.vector.tensor_copy(osb[:], psos[mb][:])
                    nc.sync.dma_start(
                        outp2[(tt16 - 12) * P:(tt16 - 11) * P,
                              mb * 512:(mb + 1) * 512],
                        osb[:],
                    )

        # ---------------- schedule ----------------
        # filler units emitted after segment s's chunk interleave; each unit
        # is needed by the s_chunks of segment s+2 (emitted during s+1) or by
        # the av chunks of the next query-block round
        FILL = {
            0: [("qk", 6, 0), ("qk", 2, 0)],
            1: [("qk", 7, 0), ("qk", 3, 0)],
            2: [("qk", 4, 1), ("qk", 0, 1), ("v", 4), ("v", 5)],
            3: [("qk", 5, 1), ("qk", 1, 1), ("v", 6), ("v", 7)],
            4: [("qk", 6, 1), ("qk", 2, 1)],
            5: [("qk", 7, 1), ("qk", 3, 1)],
            6: [("qk", 4, 2), ("qk", 0, 2), ("v", 8), ("v", 9)],
            7: [("qk", 5, 2), ("qk", 1, 2), ("v", 10), ("v", 11)],
            8: [("qk", 6, 2), ("qk", 2, 2)],
            9: [("qk", 7, 2), ("qk", 3, 2)],
            10: [("qk", 4, 3), ("qk", 0, 3), ("v", 12), ("v", 13)],
            11: [("qk", 5, 3), ("qk", 1, 3), ("v", 14), ("v", 15)],
            12: [("qk", 6, 3), ("qk", 2, 3)],
            13: [("qk", 7, 3), ("qk", 3, 3)],
        }

        def unit_gen(u):
            if u[0] == "qk":
                return qk_unit_gen(u[1], u[2])
            return v_unit_gen(u[1])

        def seg_hp_tb(s):
            # tb-major: all head pairs finish query-block tb before tb+1, so
            # each o-proj group fires early and overlaps subsequent attention
            return s % 4, s // 4

        norm_pending = []
        normed = set()

        def flush_norm():
            seg, hp, tb = norm_pending.pop(0)
            norm_seg(seg)
            normed.add(seg)
            if all((h * NTT + tb) in normed for h in range(4)):
                if tb == 3:
                    oproj3_partB()
                else:
                    oproj_group(tb)

        # prologue: enough q/k/v for segments 0 and 1, scores of segment 0
        qk_unit(4, 0)
        qk_unit(0, 0)
        for j in range(4):
            s_chunk(0, 0, j)
        v_unit(0)
        v_unit(1)
        qk_unit(5, 0)
        qk_unit(1, 0)
        v_unit(2)
        v_unit(3)

        for s in range(16):
            hp, tb = seg_hp_tb(s)
            psav = [
                ps2.tile([P, 512], f32, tag="psav", bufs=2,
                         name=f"psav_{hp}_{tb}_{i}")
                for i in range(2)
            ]
            js_a = list(range(4 * tb + 4))
            if s + 1 < 16:
                nhp, ntb = seg_hp_tb(s + 1)
                js_s = list(range(4 * ntb + 4))
            else:
                js_s = []
            nslots = max(len(js_a), len(js_s))
            for m in range(nslots):
                if m < len(js_s):
                    s_chunk(nhp, ntb, js_s[m])
                if m < len(js_a):
                    av_chunk(hp, tb, js_a[m], psav)
                if s == 15 and m in (2, 5, 8, 11):
                    # interleave the last group's hc=0..2 projection with
                    # segment 15's AV; needs segs 3, 7, 11 normalized,
                    # guaranteed by the aggressive flush at s >= 13
                    oproj3_partA(12 + (2, 5, 8, 11).index(m))
            for u in FILL.get(s, []):
                for _ in unit_gen(u):
                    pass
            norm_pending.append((hp * NTT + tb, hp, tb))
            if s >= 13:
                # drain everything whose AV is already complete, so segs 3, 7
                # and 11 are normalized before partA needs them
                while norm_pending and norm_pending[0][0] != 15:
                    flush_norm()
            elif len(norm_pending) > 2:
                flush_norm()
        while norm_pending:
            flush_norm()


_cached = {}


def build_program(split=True, ncopies=1):
    key = ("nc", ncopies)
    if key not in _cached:
        import concourse.bass as bass
        import concourse.tile as tile

        nc = bass.Bass("TRN2", target_bir_lowering=False, debug=False)
        with tile.TileContext(nc) as tc:
            aps = _declare(nc)
            for _ in range(ncopies):
                _emit(nc, tc, aps)
        _cached[key] = nc
    if split and not _cached.get(("split", ncopies)):
        _split_waits(_cached[key])
        _cached[("split", ncopies)] = True
    return _cached[key]


def make_in_maps(x, W_qkv, b_qkv, W_o):
    x = np.asarray(x, dtype=np.float32)
    W_qkv = np.asarray(W_qkv, dtype=np.float32)
    b_qkv = np.asarray(b_qkv, dtype=np.float32)
    W_o = np.asarray(W_o, dtype=np.float32)
    maskT = np.triu(np.ones((P, P), np.float32)).astype(BF16)
    ones1 = np.ones((1, 64), BF16)
    in_maps = []
    for core in range(NCORES):
        b, g = core // 2, core % 2
        qs = slice(g * CQ, (g + 1) * CQ)
        xTc = np.ascontiguousarray(x[b].T).astype(BF16)
        wq = W_qkv[:, 0:C][:, qs]
        wk = W_qkv[:, C:2 * C][:, qs]
        wvs = np.ascontiguousarray(W_qkv[:, 2 * C:3 * C][:, qs]).astype(BF16)
        wqks = np.ascontiguousarray(np.concatenate([wq, wk], axis=1)).astype(BF16)
        bqv = b_qkv[0:C][qs]
        bq_t = np.ascontiguousarray(bqv.reshape(4, P).T).astype(np.float32)
        wos = np.ascontiguousarray(W_o[qs, :]).astype(BF16)
        in_maps.append(
            dict(xT=xTc, wqk=wqks, bq=bq_t, wv=wvs, wo=wos,
                 maskT=maskT, ones1=ones1)
        )
    return in_maps


def run(x, W_qkv, b_qkv, W_o, b_o, trace=False, trace_kwargs=None):
    import time as _time

    from concourse.bass_utils import run_bass_kernel_spmd

    nc = build_program()
    in_maps = make_in_maps(x, W_qkv, b_qkv, W_o)
    last_err = None
    for attempt in range(3):
        try:
            res = run_bass_kernel_spmd(
                nc, in_maps, core_ids=list(range(NCORES)), trace=trace,
                **(trace_kwargs or {}),
            )
            break
        except Exception as e:  # transient device wedge -> retry
            last_err = e
            _time.sleep(5)
    else:
        raise last_err
    b_qkv = np.asarray(b_qkv, dtype=np.float32)
    W_o = np.asarray(W_o, dtype=np.float32)
    # v-bias contributes b_v @ W_o to every output row; k-bias cancels in the
    # softmax; both are folded into the host-side bias add.
    b_eff = np.asarray(b_o, dtype=np.float32) + b_qkv[2 * C:3 * C] @ W_o
    out = np.empty((B, T, C), np.float32)
    for b in range(B):
        out[b] = (res.results[2 * b]["outp"].astype(np.float32)
                  + res.results[2 * b + 1]["outp"].astype(np.float32) + b_eff)
        # hc=3 contribution of the last query-block group, shipped separately
        out[b][3 * 512:] += (res.results[2 * b]["outp2"].astype(np.float32)
                             + res.results[2 * b + 1]["outp2"].astype(np.float32))
    return out, res


def kernel(x, W_qkv, b_qkv, W_o, b_o):
    out, _ = run(x, W_qkv, b_qkv, W_o, b_o, trace=False)
    return out


# revision 41
# speedup vs baseline: 1.0048x; 1.0048x over previous
"""Multi-head causal attention (B=4, T=2048, C=1024, H=16) on 8 TRN2 cores.

Sharding: core i handles batch b = i//2 and head-group g = i%2 (8 heads each).
Each core computes qkv projection for its heads, causal attention, and a
partial output projection (its heads' rows of W_o). The host sums the two
partials per batch and adds b_o (plus b_v @ W_o — the v-bias contribution is
linear and folded into the host-side bias; the k-bias is dropped entirely
since a per-query multiplicative factor cancels in the softmax).

Device kernel (per core, same SPMD program), all matmuls bf16 with fp32 PSUM.
Single merged pipeline: the qkv/o projections are emitted as PE "filler"
units interleaved between attention segments, so the scalar engine (exp) gets
work from ~8us onward and the tensor engine never idles between phases.

  - qkT = (Wqk^T x^T) transposed [1024 feats, 2048] computed in 32 units of
    (nt feature-block, tt 512-query-block), each 8 cc-chunk matmuls; q units
    get the q-bias on DVE, k units are plain copies
  - v = x Wv natural [2048, 512] in 16 per-t-tile units
  - per head pair (2hp, 2hp+1), per 512-wide query block tb, per 128-wide key
    chunk j (causal-trimmed):
      S^T = K^T q^T   [128 tk, tq]  (row-packed pairs, K=64 contraction)
      P^T = exp(S^T * 0.125)        (ACT; diagonal chunks masked on Pool)
      AV^T += [V | 1]^T P^T         [65, tq]  (row 64 = softmax denominator)
    scores for segment s+1 are chunk-interleaved with AV of segment s;
    reciprocals fire straight off the PSUM denominator row at AV-complete;
    normalization (ones-outer-product broadcast on PE, then a fused
    PSUM-direct multiply on DVE) lags two segments
  - out_part = attT^T W_o rows [2048, 1024], per query-block group as soon as
    its last head pair normalizes; PSUM drained on DVE, stored bf16
  - PSUM: score pipeline 2x[128,1024] (4 banks), AV accumulators 2x[128,512]
    (2 banks), shared fill tag for proj/oproj/norm 2x[128,512] (2 banks)
"""

import sys

sys.path.insert(0, "/opt/trn_rl_repo")

import numpy as np
import ml_dtypes

BF16 = ml_dtypes.bfloat16

B, T, C, H, D = 4, 2048, 1024, 16, 64
HPC = 8        # heads per core
CQ = HPC * D   # 512
NCORES = 8
P = 128


def _split_waits(nc):
    """This container's walrus accepts only ONE sync wait per instruction.
    Split any instruction carrying N>1 waits into N-1 single-wait NoOps on
    the same engine immediately before it."""
    import concourse.mybir as mybir

    ctr = 0
    for fn in nc.m.functions:
        for bb in fn.blocks:
            insts = list(bb.instructions)
            new_insts = []
            changed = False
            for inst in insts:
                si = inst.sync_info
                if si is not None and si.on_wait and len(si.on_wait) > 1:
                    waits = list(si.on_wait)
                    for w in waits[:-1]:
                        ctr += 1
                        nop = mybir.InstNoOp(
                            name=f"I-wsplit-{ctr}",
                            engine=inst.engine,
                            ins=[],
                            outs=[],
                            sync_info=mybir.SyncInfo(on_wait=[w], on_update=[]),
                        )
                        new_insts.append(nop)
                    si.on_wait = [waits[-1]]
                    changed = True
                new_insts.append(inst)
            if changed:
                bb.instructions[:] = new_insts
    return ctr


def _declare(nc):
    import concourse.mybir as mybir

    bf = mybir.dt.bfloat16
    f32 = mybir.dt.float32
    return dict(
        xT=nc.dram_tensor("xT", [C, T], bf, kind="ExternalInput").ap(),
        wqk=nc.dram_tensor("wqk", [C, 2 * CQ], bf, kind="ExternalInput").ap(),
        bq=nc.dram_tensor("bq", [P, 4], f32, kind="ExternalInput").ap(),
        wv=nc.dram_tensor("wv", [C, CQ], bf, kind="ExternalInput").ap(),
        wo=nc.dram_tensor("wo", [CQ, C], bf, kind="ExternalInput").ap(),
        maskT=nc.dram_tensor("maskT", [P, P], bf, kind="ExternalInput").ap(),
        ones1=nc.dram_tensor("ones1", [1, 64], bf, kind="ExternalInput").ap(),
        outp=nc.dram_tensor("outp", [T, C], bf, kind="ExternalOutput").ap(),
        # hc=3 contribution of the last query-block group (rows 1536:2048);
        # the host adds it — keeps the kernel tail free of partial-add chains
        outp2=nc.dram_tensor("outp2", [512, C], bf, kind="ExternalOutput").ap(),
    )


def _emit(nc, tc, aps):
    import concourse.mybir as mybir
    from concourse.alu_op_type import AluOpType

    bf = mybir.dt.bfloat16
    f32 = mybir.dt.float32
    Exp = mybir.ActivationFunctionType.Exp

    xT = aps["xT"]; wqk = aps["wqk"]; bq = aps["bq"]; wv = aps["wv"]
    wo = aps["wo"]; maskT = aps["maskT"]; ones1 = aps["ones1"]
    outp = aps["outp"]; outp2 = aps["outp2"]

    NTT = T // 512  # 4 query blocks
    VW = HPC * 65   # 520: v row layout (64 cols + ones col per head)

    with tc.tile_pool(name="const", bufs=1) as cpool, \
         tc.tile_pool(name="proj", bufs=1) as ppool, \
         tc.tile_pool(name="ps2", bufs=1, space="PSUM") as ps2, \
         tc.tile_pool(name="ps4", bufs=1, space="PSUM") as ps4, \
         tc.tile_pool(name="att", bufs=1) as apool, \
         tc.tile_pool(name="out", bufs=1) as opool:

        wo_sb = cpool.tile([P, 4 * 1024], bf)
        maskT_sb = cpool.tile([P, P], bf)
        ones1_sb = cpool.tile([1, 64], bf)
        qkT_sb = cpool.tile([P, 8 * T], bf)
        v_sb = cpool.tile([P, 16 * VW], bf)
        attn_sb = cpool.tile([P, 16 * 512], bf)     # unnorm AV^T, pair layout

        xT_sb = ppool.tile([P, 8 * T], bf)
        wqk_sb = ppool.tile([P, 8 * 1024], bf)
        wv_sb = ppool.tile([P, 8 * CQ], bf)
        bq_sb = ppool.tile([P, 4], f32)

        # DMA order: what the first proj units need comes first — k-half of
        # wqk + first query-quarter of xT (unit qk(4,0)), then the q-half
        # (unit qk(0,0)), then wv (v units follow the first s_chunks), then
        # the remaining xT quarters and wo. One rearranged descriptor per
        # logical block: HWDGE per-descriptor overhead dominates the startup
        # window otherwise.
        wqk_v = wqk_sb.rearrange("p (c n) -> p c n", c=8)
        wqk_d = wqk.rearrange("(c p) n -> p c n", p=P)
        xT_v = xT_sb.rearrange("p (c n) -> p c n", c=8)
        xT_d = xT.rearrange("(c p) n -> p c n", p=P)
        nc.sync.dma_start(wqk_v[:, :, 512:640], wqk_d[:, :, 512:640])   # nt4
        for c4 in range(4):
            nc.sync.dma_start(xT_v[:, 2 * c4:2 * c4 + 2, 0:512],
                              xT_d[:, 2 * c4:2 * c4 + 2, 0:512])
        nc.sync.dma_start(wqk_v[:, :, 0:128], wqk_d[:, :, 0:128])       # nt0
        nc.sync.dma_start(bq_sb[:], bq[:])
        nc.sync.dma_start(wqk_v[:, :, 640:1024], wqk_d[:, :, 640:1024])
        nc.sync.dma_start(wqk_v[:, :, 128:512], wqk_d[:, :, 128:512])
        nc.sync.dma_start(wv_sb.rearrange("p (c n) -> p c n", c=8),
                          wv.rearrange("(c p) n -> p c n", p=P))
        nc.sync.dma_start(maskT_sb[:], maskT[:])
        nc.sync.dma_start(ones1_sb[:], ones1[:])
        nc.sync.dma_start(xT_v[:, :, 512:2048], xT_d[:, :, 512:2048])
        nc.sync.dma_start(wo_sb.rearrange("p (h n) -> p h n", h=4),
                          wo.rearrange("(h p) n -> p h n", p=P))

        # ones columns of v (col 64 of each head's 65-wide strip)
        v_ones = v_sb.rearrange("p (a c) -> p a c", c=65)
        nc.gpsimd.memset(v_ones[:, :, 64:65], 1.0)

        # ---------------- filler units (projections) ----------------
        # generator form: one cc-chunk matmul per next(), so units can be
        # spread across the attention chunk slots — the extra PE work between
        # successive score chunks hides the scores->exp->AV semaphore
        # round-trip that would otherwise stall PE on the 2-deep score PSUM
        # rotation
        def qk_unit_gen(nt, tt):
            psq = ps4.tile([P, 512], f32, tag="fill", bufs=2,
                           name=f"psq_{nt}_{tt}")
            for cc in range(8):
                nc.tensor.matmul(
                    psq[:],
                    wqk_sb[:, cc * 1024 + nt * P: cc * 1024 + (nt + 1) * P],
                    xT_sb[:, cc * T + tt * 512: cc * T + (tt + 1) * 512],
                    start=(cc == 0),
                    stop=(cc == 7),
                )
                if cc < 7:
                    yield
            dst = qkT_sb[:, nt * T + tt * 512: nt * T + (tt + 1) * 512]
            if nt < 4:
                nc.vector.tensor_scalar(dst, psq[:], bq_sb[:, nt:nt + 1],
                                        None, op0=AluOpType.add)
            else:
                nc.vector.tensor_copy(dst, psq[:])

        def v_unit_gen(tt16):
            psv = ps4.tile([P, 512], f32, tag="fill", bufs=2,
                           name=f"psv_{tt16}")
            for cc in range(8):
                nc.tensor.matmul(
                    psv[:],
                    xT_sb[:, cc * T + tt16 * P: cc * T + (tt16 + 1) * P],
                    wv_sb[:, cc * CQ:(cc + 1) * CQ],
                    start=(cc == 0),
                    stop=(cc == 7),
                )
                if cc < 7:
                    yield
            vv = v_sb[:, tt16 * VW:(tt16 + 1) * VW].rearrange(
                "p (a c) -> p a c", c=65)
            nc.vector.tensor_copy(
                vv[:, :, 0:64], psv[:].rearrange("p (a c) -> p a c", c=64))

        def qk_unit(nt, tt):
            for _ in qk_unit_gen(nt, tt):
                pass

        def v_unit(tt16):
            for _ in v_unit_gen(tt16):
                pass

        # ---------------- attention ----------------
        pts = {}
        recs = {}

        def s_chunk(hp, tb, j):
            h0, h1 = 2 * hp, 2 * hp + 1
            off = j * P - tb * 512
            nstart = max(off, 0)
            pss = ps2.tile([P, 1024], f32, tag="pss", bufs=2,
                           name=f"pss_{hp}_{tb}_{j}")
            pt = apool.tile([P, 1024], bf, tag="pt", bufs=24,
                            name=f"pt_{hp}_{tb}_{j}")
            pts[(hp, tb, j)] = pt
            for i, hl in enumerate((h0, h1)):
                base = (hl % 2) * 64
                nc.tensor.matmul(
                    pss[:, i * 512 + nstart: i * 512 + 512],
                    qkT_sb[base:base + 64,
                           (4 + hp) * T + j * P: (4 + hp) * T + (j + 1) * P],
                    qkT_sb[base:base + 64,
                           hp * T + tb * 512 + nstart: hp * T + (tb + 1) * 512],
                    start=True,
                    stop=True,
                )
            pw = pss.rearrange("p (a c) -> p a c", c=512)
            ptw = pt.rearrange("p (a c) -> p a c", c=512)
            nc.scalar.activation(
                ptw[:, :, nstart:512], pw[:, :, nstart:512], Exp, scale=0.125,
            )
            if off >= 0:
                for i in range(2):
                    nc.gpsimd.tensor_tensor(
                        pt[:, i * 512 + nstart: i * 512 + nstart + P],
                        pt[:, i * 512 + nstart: i * 512 + nstart + P],
                        maskT_sb[:],
                        op=AluOpType.mult,
                    )

        def av_chunk(hp, tb, j, psav):
            h0, h1 = 2 * hp, 2 * hp + 1
            jmax = 4 * tb + 3
            off = j * P - tb * 512
            nstart = max(off, 0)
            for i, hl in enumerate((h0, h1)):
                nc.tensor.matmul(
                    psav[i][0:65, nstart:512],
                    v_sb[:, j * VW + hl * 65: j * VW + (hl + 1) * 65],
                    pts[(hp, tb, j)][:, i * 512 + nstart: i * 512 + 512],
                    start=(j == 0),
                    stop=(j == jmax),
                )
            if j == jmax:
                seg = hp * NTT + tb
                rece = apool.tile([1, 512], bf, tag="rec", bufs=8,
                                  name=f"rece_{seg}")
                reco = apool.tile([1, 512], bf, tag="rec", bufs=8,
                                  name=f"reco_{seg}")
                with nc.allow_low_precision(reason="bf16 softmax denominators"):
                    nc.vector.reciprocal(rece[:], psav[0][64:65, :])
                    nc.vector.reciprocal(reco[:], psav[1][64:65, :])
                recs[seg] = (rece, reco)
                nc.vector.tensor_copy(attn_sb[0:64, seg * 512:(seg + 1) * 512],
                                      psav[0][0:64, :])
                nc.vector.tensor_copy(attn_sb[64:128, seg * 512:(seg + 1) * 512],
                                      psav[1][0:64, :])

        def norm_seg(seg):
            # denominator reciprocals broadcast across partitions on PE (ones
            # outer product), then applied straight from PSUM on DVE — keeps
            # the normalize off Pool, whose in-order queue stalls on
            # exp-gated mask multiplies
            sl = slice(seg * 512, (seg + 1) * 512)
            rece, reco = recs.pop(seg)
            if seg == 15:
                # tail: score banks are idle, avoid the fill-tag rotation
                psr = ps2.tile([P, 1024], f32, tag="pss", bufs=2,
                               name=f"psr_{seg}")[:, 0:512]
            else:
                psr = ps4.tile([P, 512], f32, tag="fill", bufs=2,
                               name=f"psr_{seg}")
            nc.tensor.matmul(psr[0:64, :], ones1_sb[:], rece[:],
                             start=True, stop=True)
            nc.tensor.matmul(psr[64:128, :], ones1_sb[:], reco[:],
                             start=True, stop=True)
            nc.vector.scalar_tensor_tensor(
                attn_sb[0:64, sl], psr[0:64, :], 1.0, attn_sb[0:64, sl],
                op0=AluOpType.bypass, op1=AluOpType.mult)
            nc.vector.scalar_tensor_tensor(
                attn_sb[64:128, sl], psr[64:128, :], 1.0, attn_sb[64:128, sl],
                op0=AluOpType.bypass, op1=AluOpType.mult)

        def oproj_group(tbg):
            # output projection for query rows tbg*512 .. +512 (4 t-tiles)
            for tt16 in range(tbg * 4, tbg * 4 + 4):
                psos = [
                    ps4.tile([P, 512], f32, tag="fill", bufs=2,
                             name=f"pso_{tt16}_{mb}")
                    for mb in range(2)
                ]
                for hc in range(4):
                    seg = hc * NTT + tbg
                    col = (seg * 4 + tt16 % 4) * P
                    for mb in range(2):
                        nc.tensor.matmul(
                            psos[mb][:],
                            attn_sb[:, col: col + P],
                            wo_sb[:, hc * 1024 + mb * 512: hc * 1024 + (mb + 1) * 512],
                            start=(hc == 0),
                            stop=(hc == 3),
                        )
                for mb in range(2):
                    osb = opool.tile([P, 512], bf, tag="osb", bufs=4,
                                     name=f"osb_{tt16}_{mb}")
                    nc.vector.tensor_copy(osb[:], psos[mb][:])
                    nc.sync.dma_start(
                        outp[tt16 * P:(tt16 + 1) * P, mb * 512:(mb + 1) * 512],
                        osb[:],
                    )

        # The last group's projection would otherwise sit entirely in the
        # kernel tail (it needs the very last segment normalized). Split it:
        # head-chunks 0..2 go straight to outp while segment 15's AV still
        # runs; the tail does only the hc=3 matmuls, whose result is shipped
        # as a separate output for the host to add.
        def oproj3_partA(tt16):
            psos = [
                ps4.tile([P, 512], f32, tag="fill", bufs=2,
                         name=f"psoA_{tt16}_{mb}")
                for mb in range(2)
            ]
            for hc in range(3):
                seg = hc * NTT + 3
                col = (seg * 4 + tt16 % 4) * P
                for mb in range(2):
                    nc.tensor.matmul(
                        psos[mb][:],
                        attn_sb[:, col: col + P],
                        wo_sb[:, hc * 1024 + mb * 512: hc * 1024 + (mb + 1) * 512],
                        start=(hc == 0),
                        stop=(hc == 2),
                    )
            for mb in range(2):
                osb = opool.tile([P, 512], bf, tag="osb", bufs=4,
                                 name=f"osbA_{tt16}_{mb}")
                nc.vector.tensor_copy(osb[:], psos[mb][:])
                nc.sync.dma_start(
                    outp[tt16 * P:(tt16 + 1) * P, mb * 512:(mb + 1) * 512],
                    osb[:],
                )

        def oproj3_partB():
            # runs in the kernel tail: use the psav banks (free once segment
            # 15's AV is drained) so the chain doesn't contend with the fill
            # tag's rotation
            for tt16 in range(12, 16):
                psos = [
                    ps2.tile([P, 512], f32, tag="psav", bufs=2,
                             name=f"psoB_{tt16}_{mb}")
                    for mb in range(2)
                ]
                seg = 3 * NTT + 3
                col = (seg * 4 + tt16 % 4) * P
                for mb in range(2):
                    nc.tensor.matmul(
                        psos[mb][:],
                        attn_sb[:, col: col + P],
                        wo_sb[:, 3 * 1024 + mb * 512: 3 * 1024 + (mb + 1) * 512],
                        start=True,
                        stop=True,
                    )
                for mb in range(2):
                    osb = opool.tile([P, 512], bf, tag="osb", bufs=4,
                                     name=f"osbB_{tt16}_{mb}")
                    nc.vector.tensor_copy(osb[:], psos[mb][:])
                    nc.sync.dma_start(
                        outp2[(tt16 - 12) * P:(tt16 - 11) * P,
                              mb * 512:(mb + 1) * 512],
                        osb[:],
                    )

        # ---------------- schedule ----------------
        # filler units emitted after segment s's chunk interleave; each unit
        # is needed by the s_chunks of segment s+2 (emitted during s+1) or by
        # the av chunks of the next query-block round
        FILL = {
            0: [("qk", 6, 0), ("qk", 2, 0)],
            1: [("qk", 7, 0), ("qk", 3, 0)],
            2: [("qk", 4, 1), ("qk", 0, 1), ("v", 4), ("v", 5)],
            3: [("qk", 5, 1), ("qk", 1, 1), ("v", 6), ("v", 7)],
            4: [("qk", 6, 1), ("qk", 2, 1)],
            5: [("qk", 7, 1), ("qk", 3, 1)],
            6: [("qk", 4, 2), ("qk", 0, 2), ("v", 8), ("v", 9)],
            7: [("qk", 5, 2), ("qk", 1, 2), ("v", 10), ("v", 11)],
            8: [("qk", 6, 2), ("qk", 2, 2)],
            9: [("qk", 7, 2), ("qk", 3, 2)],
            10: [("qk", 4, 3), ("qk", 0, 3), ("v", 12), ("v", 13)],
            11: [("qk", 5, 3), ("qk", 1, 3), ("v", 14), ("v", 15)],
            12: [("qk", 6, 3), ("qk", 2, 3)],
            13: [("qk", 7, 3), ("qk", 3, 3)],
        }

        def unit_gen(u):
            if u[0] == "qk":
                return qk_unit_gen(u[1], u[2])
            return v_unit_gen(u[1])

        def seg_hp_tb(s):
            # tb-major: all head pairs finish query-block tb before tb+1, so
            # each o-proj group fires early and overlaps subsequent attention
            return s % 4, s // 4

        norm_pending = []
        normed = set()

        def flush_norm():
            seg, hp, tb = norm_pending.pop(0)
            norm_seg(seg)
            normed.add(seg)
            if all((h * NTT + tb) in normed for h in range(4)):
                if tb == 3:
                    oproj3_partB()
                else:
                    oproj_group(tb)

        # prologue: enough q/k/v for segments 0 and 1, scores of segment 0
        qk_unit(4, 0)
        qk_unit(0, 0)
        for j in range(4):
            s_chunk(0, 0, j)
        v_unit(0)
        v_unit(1)
        qk_unit(5, 0)
        qk_unit(1, 0)
        v_unit(2)
        v_unit(3)

        for s in range(16):
            hp, tb = seg_hp_tb(s)
            psav = [
                ps2.tile([P, 512], f32, tag="psav", bufs=2,
                         name=f"psav_{hp}_{tb}_{i}")
                for i in range(2)
            ]
            js_a = list(range(4 * tb + 4))
            if s + 1 < 16:
                nhp, ntb = seg_hp_tb(s + 1)
                js_s = list(range(4 * ntb + 4))
            else:
                js_s = []
            nslots = max(len(js_a), len(js_s))
            for m in range(nslots):
                if m < len(js_a):
                    av_chunk(hp, tb, js_a[m], psav)
                if m < len(js_s):
                    s_chunk(nhp, ntb, js_s[m])
                if s == 15 and m in (2, 5, 8, 11):
                    # interleave the last group's hc=0..2 projection with
                    # segment 15's AV; needs segs 3, 7, 11 normalized,
                    # guaranteed by the aggressive flush at s >= 13
                    oproj3_partA(12 + (2, 5, 8, 11).index(m))
            for u in FILL.get(s, []):
                for _ in unit_gen(u):
                    pass
            norm_pending.append((hp * NTT + tb, hp, tb))
            if s >= 13:
                # drain everything whose AV is already complete, so segs 3, 7
                # and 11 are normalized before partA needs them
                while norm_pending and norm_pending[0][0] != 15:
                    flush_norm()
            elif len(norm_pending) > 2:
                flush_norm()
        while norm_pending:
            flush_norm()


_cached = {}


def build_program(split=True, ncopies=1):
    key = ("nc", ncopies)
    if key not in _cached:
        import concourse.bass as bass
        import concourse.tile as tile

        nc = bass.Bass("TRN2", target_bir_lowering=False, debug=False)
        with tile.TileContext(nc) as tc:
            aps = _declare(nc)
            for _ in range(ncopies):
                _emit(nc, tc, aps)
        _cached[key] = nc
    if split and not _cached.get(("split", ncopies)):
        _split_waits(_cached[key])
        _cached[("split", ncopies)] = True
    return _cached[key]


def make_in_maps(x, W_qkv, b_qkv, W_o):
    x = np.asarray(x, dtype=np.float32)
    W_qkv = np.asarray(W_qkv, dtype=np.float32)
    b_qkv = np.asarray(b_qkv, dtype=np.float32)
    W_o = np.asarray(W_o, dtype=np.float32)
    maskT = np.triu(np.ones((P, P), np.float32)).astype(BF16)
    ones1 = np.ones((1, 64), BF16)
    in_maps = []
    for core in range(NCORES):
        b, g = core // 2, core % 2
        qs = slice(g * CQ, (g + 1) * CQ)
        xTc = np.ascontiguousarray(x[b].T).astype(BF16)
        wq = W_qkv[:, 0:C][:, qs]
        wk = W_qkv[:, C:2 * C][:, qs]
        wvs = np.ascontiguousarray(W_qkv[:, 2 * C:3 * C][:, qs]).astype(BF16)
        wqks = np.ascontiguousarray(np.concatenate([wq, wk], axis=1)).astype(BF16)
        bqv = b_qkv[0:C][qs]
        bq_t = np.ascontiguousarray(bqv.reshape(4, P).T).astype(np.float32)
        wos = np.ascontiguousarray(W_o[qs, :]).astype(BF16)
        in_maps.append(
            dict(xT=xTc, wqk=wqks, bq=bq_t, wv=wvs, wo=wos,
                 maskT=maskT, ones1=ones1)
        )
    return in_maps


def run(x, W_qkv, b_qkv, W_o, b_o, trace=False, trace_kwargs=None):
    import time as _time

    from concourse.bass_utils import run_bass_kernel_spmd

    nc = build_program()
    in_maps = make_in_maps(x, W_qkv, b_qkv, W_o)
    last_err = None
    for attempt in range(3):
        try:
            res = run_bass_kernel_spmd(
                nc, in_maps, core_ids=list(range(NCORES)), trace=trace,
                **(trace_kwargs or {}),
            )
            break
        except Exception as e:  # transient device wedge -> retry
            last_err = e
            _time.sleep(5)
    else:
        raise last_err
    b_qkv = np.asarray(b_qkv, dtype=np.float32)
    W_o = np.asarray(W_o, dtype=np.float32)
    # v-bias contributes b_v @ W_o to every output row; k-bias cancels in the
    # softmax; both are folded into the host-side bias add.
    b_eff = np.asarray(b_o, dtype=np.float32) + b_qkv[2 * C:3 * C] @ W_o
    out = np.empty((B, T, C), np.float32)
    for b in range(B):
        out[b] = (res.results[2 * b]["outp"].astype(np.float32)
                  + res.results[2 * b + 1]["outp"].astype(np.float32) + b_eff)
        # hc=3 contribution of the last query-block group, shipped separately
        out[b][3 * 512:] += (res.results[2 * b]["outp2"].astype(np.float32)
                             + res.results[2 * b + 1]["outp2"].astype(np.float32))
    return out, res


def kernel(x, W_qkv, b_qkv, W_o, b_o):
    out, _ = run(x, W_qkv, b_qkv, W_o, b_o, trace=False)
    return out
